# revision 6
# baseline (speedup 1.0000x reference)
"""Trainium2 Bass kernel for nn_CustomLoss (cross-entropy + worst-class masked loss).

Computes: loss = mean_i(logsumexp(output_i) - output_i[target_i])
          result = loss * (1 + mean_i(target_i in {3,5,8,9}))

Data-parallel over 8 NeuronCores. The device is a pure streaming-logsumexp
machine: each core reads its 32768x1000 shard (quantized on host: fp8e4m3
for ACT tiles, bf16/fp8 for DVE tiles) and computes per-row sum(exp(x)) with
two engines in parallel:
  - ACT: exact Exp with free-dim accumulation (1 elem/cycle/lane, any dtype)
  - DVE: Schraudolph bit-trick exp: i16 = int(x*A + B) stored to int16,
    bitcast-read as bf16 == 2^(x*log2e - bias), summed via tensor_scalar
    accum_out (4x perf mode on 2-byte dtypes)
then Ln + column-sum -> [128, 2] partial lse sums per core.

Host computes the target-logit gather sum(x[i, t_i]) (0.1% of the data) and
the worst-class mask mean from the full-precision input, and combines:
  loss = (sum_lse - sum_gather)/B;  result = loss * (1 + mask_mean).

Quantization error budget (measured in numpy on N(0,1) logits): fp8 path
lse bias +1e-5, trick path bias < 1.5e-3 with C_TRICK calibrated for either
float->int rounding mode; final relative error ~2e-4 vs 2e-2 tolerance.
"""
import numpy as np
from contextlib import ExitStack

import concourse.bacc as bacc
import concourse.tile as tile
from concourse import mybir
from concourse.bass_utils import run_bass_kernel_spmd

F32 = mybir.dt.float32
BF16 = mybir.dt.bfloat16
I16 = mybir.dt.int16
F8 = mybir.dt.float8e4
AF = mybir.ActivationFunctionType
ALU = mybir.AluOpType

N_CORES = 8
B, C = 262144, 1000
ROWS = B // N_CORES           # 32768 rows per core
P = 128                       # SBUF partitions
G = 16                        # [128, C] sub-tiles per DMA chunk
N_CHUNKS = ROWS // (P * G)    # 16 chunks
K8 = 7                        # tiles per chunk on ACT (fp8, exact exp)
KV = G - K8                   # tiles per chunk on DVE (bit-trick exp)
DV_DTYPE = F8                 # dtype of the DVE-tile stream (F8 or BF16)
WORST = (3, 5, 8, 9)

# Schraudolph constants for bf16-bitspace exp: bits = x*A + (16256 - c).
# c calibrated so the mean of trick_exp/exp over N(0,1) logits is ~1; the
# DVE float->int16 store truncates (verified bit-exact against CoreSim).
A_TRICK = 128.0 / float(np.log(2.0))
B_TRICK = 16256.0 - 6.871

_CACHE = {}


def _build(reps: int = 1, k8: int = K8, dv_dtype=DV_DTYPE, g: int = G,
           n_chunks: int | None = None, x_internal: bool = False):
    kv = g - k8
    if n_chunks is None:
        n_chunks = ROWS // (P * g)
    nc = bacc.Bacc(None, target_bir_lowering=False, debug=False,
                   num_devices=N_CORES)

    def declare_x(name, shape, dtype):
        if not x_internal:
            return nc.declare_dram_parameter(name, shape, dtype, isOutput=False)
        # Timing-only builds: x as Internal DRAM (uninitialized) so repeated
        # timed calls carry no input upload. Output values are garbage.
        from concourse.bass import DRamTensorHandle
        nc._tensor(name, shape, dtype, kind="Internal", type="DRAM")
        return DRamTensorHandle(name, shape, dtype)

    xa_h = xv_h = None
    if k8 > 0:
        xa_h = declare_x("xa", [n_chunks, P, k8 * C], F8)
    if kv > 0:
        xv_h = declare_x("xv", [n_chunks, P, kv * C], dv_dtype)
    out_h = nc.declare_dram_parameter("out", [P, 2], F32, isOutput=True)

    with tile.TileContext(nc) as tc, ExitStack() as ctx:
        xap = ctx.enter_context(tc.tile_pool(name="xap", bufs=3))
        xvp = ctx.enter_context(tc.tile_pool(name="xvp", bufs=3))
        scr = ctx.enter_context(tc.tile_pool(name="scr", bufs=4))
        pers = ctx.enter_context(tc.tile_pool(name="pers", bufs=1))

        n_a, n_v = n_chunks * k8, n_chunks * kv
        s_a = pers.tile([P, max(n_a, 1)], F32, tag="s_a")
        s_v = pers.tile([P, max(n_v, 1)], F32, tag="s_v")
        fin = pers.tile([P, 2], F32, tag="fin")

        def body():
            ka = kv_i = 0
            for ch in range(n_chunks):
                xa_t = xv_t = None
                if k8 > 0:
                    xa_t = xap.tile([P, k8 * C], F8, tag="xa_t")
                    nc.sync.dma_start(out=xa_t[:], in_=xa_h[ch])
                if kv > 0:
                    xv_t = xvp.tile([P, kv * C], dv_dtype, tag="xv_t")
                    nc.sync.dma_start(out=xv_t[:], in_=xv_h[ch])
                for j in range(k8):
                    nonlocal_ka = ka + j
                    e_scr = scr.tile([P, C], BF16, tag="e_scr")
                    nc.scalar.activation(
                        out=e_scr[:], in_=xa_t[:, j * C:(j + 1) * C],
                        func=AF.Exp, accum_out=s_a[:, nonlocal_ka:nonlocal_ka + 1],
                    )
                for j in range(kv):
                    k = kv_i + j
                    i_scr = scr.tile([P, C], I16, tag="i_scr")
                    nc.vector.tensor_scalar(
                        out=i_scr[:], in0=xv_t[:, j * C:(j + 1) * C],
                        scalar1=A_TRICK, scalar2=B_TRICK,
                        op0=ALU.mult, op1=ALU.add,
                    )
                    o_scr = scr.tile([P, C], BF16, tag="o_scr")
                    # op1 is the accumulate-reduce op when accum_out is set
                    nc.vector.tensor_scalar(
                        out=o_scr[:], in0=i_scr[:].bitcast(BF16),
                        scalar1=1.0, scalar2=None, op0=ALU.mult, op1=ALU.add,
                        accum_out=s_v[:, k:k + 1],
                    )
                ka += k8
                kv_i += kv

            if n_a > 0:
                ln_a = scr.tile([P, n_a], BF16, tag="ln_a")
                nc.scalar.activation(out=ln_a[:], in_=s_a[:, :n_a], func=AF.Ln,
                                     accum_out=fin[:, 0:1])
            else:
                nc.vector.memset(fin[:, 0:1], 0.0)
            if n_v > 0:
                ln_v = scr.tile([P, n_v], BF16, tag="ln_v")
                nc.scalar.activation(out=ln_v[:], in_=s_v[:, :n_v], func=AF.Ln,
                                     accum_out=fin[:, 1:2])
            else:
                nc.vector.memset(fin[:, 1:2], 0.0)
            nc.sync.dma_start(out=out_h[:], in_=fin[:])

        if reps == 1:
            body()
        else:
            with tc.For_i(0, reps):
                body()

    nc.compile()
    return nc


def _shard_inputs(output: np.ndarray, k8: int = K8, dv_dtype=DV_DTYPE,
                  g: int = G):
    import ml_dtypes
    kv = g - k8
    n_chunks = ROWS // (P * g)
    np_dv = np.dtype(mybir.dt.np(dv_dtype))
    in_maps = []
    for c in range(N_CORES):
        xs = output[c * ROWS:(c + 1) * ROWS]
        v = xs.reshape(n_chunks, P, g, C)
        m = {}
        if k8 > 0:
            m["xa"] = np.ascontiguousarray(
                v[:, :, :k8, :]).astype(ml_dtypes.float8_e4m3).reshape(
                    n_chunks, P, k8 * C)
        if kv > 0:
            m["xv"] = np.ascontiguousarray(
                v[:, :, k8:, :]).astype(np_dv).reshape(n_chunks, P, kv * C)
        in_maps.append(m)
    return in_maps


def _host_terms(output: np.ndarray, target: np.ndarray):
    g_sum = output[np.arange(B), target].astype(np.float64).sum()
    mask_mean = float(np.isin(target, np.asarray(WORST)).mean())
    return g_sum, mask_mean


def _combine(results, g_sum: float, mask_mean: float) -> np.float32:
    lse_sum = 0.0
    for r in results:
        lse_sum += float(r["out"].astype(np.float64).sum())
    loss = (lse_sum - g_sum) / B
    return np.float32(loss * (1.0 + mask_mean))


def _run(in_maps, **kwargs):
    if "nc" not in _CACHE:
        _CACHE["nc"] = _build()
    return run_bass_kernel_spmd(_CACHE["nc"], in_maps, list(range(N_CORES)),
                                **kwargs)


def kernel(output: np.ndarray, target: np.ndarray) -> np.float32:
    assert output.shape == (B, C) and target.shape == (B,)
    res = _run(_shard_inputs(output))
    g_sum, mask_mean = _host_terms(output, target)
    return _combine(res.results, g_sum, mask_mean)


# revision 8
# speedup vs baseline: 1.8069x; 1.8069x over previous
"""Trainium2 Bass kernel for nn_CustomLoss (cross-entropy + worst-class masked loss).

Computes: loss = mean_i(logsumexp(output_i) - output_i[target_i])
          result = loss * (1 + mean_i(target_i in {3,5,8,9}))

Data-parallel over 8 NeuronCores. The device is a pure streaming-logsumexp
machine over the fp8-quantized logits, using three engines in parallel:

  - ACT path (row-major tiles [128 rows, 1000 classes]): exact Exp with
    fused free-dim accumulation -> per-row sum(exp). ~1.25us/tile.
  - DVE+PE path (host-transposed super-tiles [128 classes, 512 rows] x 8
    class-chunks, classes padded to 1024): DVE computes the Schraudolph
    bit-trick exp in one 4096-wide tensor_scalar (i16 = int(x*A + B); the
    i16 bit pattern read as bf16 *is* ~exp(x)), then the idle TensorE sums
    over the class partitions with one-hot-column matmuls accumulating into
    PSUM[s, :] (super-tile s -> PSUM partition s). DVE ~2.2us + PE ~1.8us
    per super-tile (4 tiles), beating the 1x accum_out cap on DVE sums.
  - Final: ACT Ln over the two sum tensors with fused accumulation -> [P,2]
    partial lse sums per core; host combines.

Host computes the target-logit gather sum(x[i, t_i]) (0.1% of the data) and
the worst-class mask mean from the full-precision input:
  loss = (sum_lse - sum_gather)/B;  result = loss * (1 + mask_mean).

Error budget (validated in numpy + CoreSim): fp8 quantization is unbiased
(per-row lse std 3e-3); the bit-trick (c calibrated for the DVE's truncating
float->int16 store) biases lse < 1.5e-3; final relative error ~2.5e-4 vs the
2e-2 tolerance.
"""
import numpy as np
from contextlib import ExitStack

import concourse.bacc as bacc
import concourse.tile as tile
from concourse import mybir
from concourse.bass_utils import run_bass_kernel_spmd

F32 = mybir.dt.float32
BF16 = mybir.dt.bfloat16
I16 = mybir.dt.int16
F8 = mybir.dt.float8e4
AF = mybir.ActivationFunctionType
ALU = mybir.AluOpType

N_CORES = 8
B, C = 262144, 1000
ROWS = B // N_CORES           # 32768 rows per core
P = 128                       # SBUF partitions
CP = 1024                     # padded classes (8 chunks of 128)
R_SUP = 512                   # rows per super-tile (one PSUM bank column dim)

# Work split: N_CH_A chunks of KA [128 rows, C] tiles on ACT (exact exp),
# N_SUP transposed super-tiles of 512 rows on DVE+PE (bit-trick exp).
KA = 5
N_CH_A = 16
N_A = KA * N_CH_A             # 80 ACT tiles (10240 rows)
N_SUP = (ROWS - N_A * P) // R_SUP   # 44 super-tiles (22528 rows)
WORST = (3, 5, 8, 9)

# Schraudolph constants for bf16-bitspace exp: bits = x*A + (16256 - c).
# c calibrated for the truncating float->int16 store (CoreSim-verified).
A_TRICK = 128.0 / float(np.log(2.0))
B_TRICK = 16256.0 - 6.871
# Class padding value: exactly representable in fp8e4m3; trick bits land at
# ~1476 -> bf16 2.3e-35 ~= 0, and exp(-80) == 0 in f32, so pads add nothing.
X_PAD = -80.0

_CACHE = {}


def _build(reps: int = 1, ka: int = KA, n_ch_a: int = N_CH_A,
           n_sup: int = N_SUP, x_internal: bool = False):
    n_a = ka * n_ch_a
    nc = bacc.Bacc(None, target_bir_lowering=False, debug=False,
                   num_devices=N_CORES)

    def declare_x(name, shape, dtype):
        if not x_internal:
            return nc.declare_dram_parameter(name, shape, dtype, isOutput=False)
        # Timing-only builds: x as Internal DRAM (uninitialized) so repeated
        # timed calls carry no input upload. Output values are garbage.
        from concourse.bass import DRamTensorHandle
        nc._tensor(name, shape, dtype, kind="Internal", type="DRAM")
        return DRamTensorHandle(name, shape, dtype)

    xa_h = declare_x("xa", [n_ch_a, P, ka * C], F8) if n_a > 0 else None
    xt_h = declare_x("xt", [n_sup, P, 8 * R_SUP], F8) if n_sup > 0 else None
    out_h = nc.declare_dram_parameter("out", [P, 2], F32, isOutput=True)

    with tile.TileContext(nc) as tc, ExitStack() as ctx:
        xap = ctx.enter_context(tc.tile_pool(name="xap", bufs=3))
        xtp = ctx.enter_context(tc.tile_pool(name="xtp", bufs=3))
        scr = ctx.enter_context(tc.tile_pool(name="scr", bufs=4))
        pers = ctx.enter_context(tc.tile_pool(name="pers", bufs=1))
        pp = ctx.enter_context(tc.tile_pool(name="pp", bufs=1, space="PSUM"))

        s_a = pers.tile([P, max(n_a, 1)], F32, tag="s_a")
        fin = pers.tile([P, 2], F32, tag="fin")
        ps = pp.tile([64, R_SUP], F32, tag="ps")
        eye = pers.tile([P, 64 * 64], BF16, tag="eye")

        nc.vector.memset(fin[:], 0.0)
        # One-hot weight slabs: eye[:, i*64:(i+1)*64][p, m] = (m == i).
        # Built once on-device; slab i routes super-tile i's sum to PSUM row i.
        nc.vector.memset(eye[:], 0.0)
        for i in range(n_sup):
            nc.vector.memset(eye[:, i * 64 + i:i * 64 + i + 1], 1.0)

        def body():
            for ch in range(n_ch_a):
                xa_t = xap.tile([P, ka * C], F8, tag="xa_t")
                nc.sync.dma_start(out=xa_t[:], in_=xa_h[ch])
                for j in range(ka):
                    k = ch * ka + j
                    e_scr = scr.tile([P, C], BF16, tag="e_scr")
                    nc.scalar.activation(
                        out=e_scr[:], in_=xa_t[:, j * C:(j + 1) * C],
                        func=AF.Exp, accum_out=s_a[:, k:k + 1],
                    )
            for s in range(n_sup):
                xt_t = xtp.tile([P, 8 * R_SUP], F8, tag="xt_t")
                nc.sync.dma_start(out=xt_t[:], in_=xt_h[s])
                i_t = scr.tile([P, 8 * R_SUP], I16, tag="i_t")
                nc.vector.tensor_scalar(
                    out=i_t[:], in0=xt_t[:], scalar1=A_TRICK, scalar2=B_TRICK,
                    op0=ALU.mult, op1=ALU.add,
                )
                for c in range(8):
                    nc.tensor.matmul(
                        out=ps[:, :],
                        lhsT=eye[:, s * 64:(s + 1) * 64],
                        rhs=i_t[:, c * R_SUP:(c + 1) * R_SUP].bitcast(BF16),
                        start=(s == 0 and c == 0),
                        stop=(s == n_sup - 1 and c == 7),
                    )

            if n_a > 0:
                ln_a = scr.tile([P, n_a], BF16, tag="ln_a")
                nc.scalar.activation(out=ln_a[:], in_=s_a[:, :n_a], func=AF.Ln,
                                     accum_out=fin[:, 0:1])
            if n_sup > 0:
                ln_p = scr.tile([64, R_SUP], BF16, tag="ln_p")
                nc.scalar.activation(out=ln_p[:n_sup], in_=ps[:n_sup, :],
                                     func=AF.Ln, accum_out=fin[:n_sup, 1:2])
            nc.sync.dma_start(out=out_h[:], in_=fin[:])

        if reps == 1:
            body()
        else:
            with tc.For_i(0, reps):
                body()

    nc.compile()
    return nc


def _shard_core(xs: np.ndarray, ka: int = KA, n_ch_a: int = N_CH_A,
                n_sup: int = N_SUP):
    """xs: [rows, C] f32 for one core -> {'xa': fp8 row-major, 'xt': fp8
    transposed super-tiles}."""
    import ml_dtypes
    f8 = ml_dtypes.float8_e4m3
    n_a = ka * n_ch_a
    rows_a = n_a * P
    m = {}
    if n_a > 0:
        m["xa"] = np.ascontiguousarray(
            xs[:rows_a].reshape(n_ch_a, P, ka * C)).astype(f8)
    if n_sup > 0:
        blocks = xs[rows_a:rows_a + n_sup * R_SUP].reshape(n_sup, R_SUP, C)
        padded = np.full((n_sup, R_SUP, CP), X_PAD, np.float32)
        padded[:, :, :C] = blocks
        # [s, r, cls] -> [s, p, c*R_SUP + r] with cls = c*128 + p
        xt = padded.transpose(0, 2, 1).reshape(n_sup, 8, P, R_SUP)
        xt = np.ascontiguousarray(xt.transpose(0, 2, 1, 3)).reshape(
            n_sup, P, 8 * R_SUP)
        m["xt"] = xt.astype(f8)
    return m


def _shard_inputs(output: np.ndarray):
    return [_shard_core(output[c * ROWS:(c + 1) * ROWS])
            for c in range(N_CORES)]


def _host_terms(output: np.ndarray, target: np.ndarray):
    g_sum = output[np.arange(B), target].astype(np.float64).sum()
    mask_mean = float(np.isin(target, np.asarray(WORST)).mean())
    return g_sum, mask_mean


def _combine(results, g_sum: float, mask_mean: float) -> np.float32:
    lse_sum = 0.0
    for r in results:
        fin = r["out"].astype(np.float64)
        lse_sum += fin[:, 0].sum() + fin[:N_SUP, 1].sum()
    loss = (lse_sum - g_sum) / B
    return np.float32(loss * (1.0 + mask_mean))


def _run(in_maps, **kwargs):
    if "nc" not in _CACHE:
        _CACHE["nc"] = _build()
    return run_bass_kernel_spmd(_CACHE["nc"], in_maps, list(range(N_CORES)),
                                **kwargs)


def kernel(output: np.ndarray, target: np.ndarray) -> np.float32:
    assert output.shape == (B, C) and target.shape == (B,)
    res = _run(_shard_inputs(output))
    g_sum, mask_mean = _host_terms(output, target)
    return _combine(res.results, g_sum, mask_mean)


# revision 11
# speedup vs baseline: 2.2739x; 1.2584x over previous
"""Trainium2 Bass kernel for nn_CustomLoss (cross-entropy + worst-class masked loss).

Computes: loss = mean_i(logsumexp(output_i) - output_i[target_i])
          result = loss * (1 + mean_i(target_i in {3,5,8,9}))

Data-parallel over 8 NeuronCores. The device is a pure streaming-logsumexp
machine over the fp8-quantized logits, using three engines in parallel:

  - ACT path (row-major tiles [128 rows, 1000 classes]): exact Exp with
    fused free-dim accumulation -> per-row sum(exp). ~1.25us/tile.
  - DVE+PE path (host-transposed super-tiles [128 classes, 512 rows] x 8
    class-chunks, classes padded to 1024): DVE computes the Schraudolph
    bit-trick exp in one 4096-wide tensor_scalar (i16 = int(x*A + B); the
    i16 bit pattern read as bf16 *is* ~exp(x)), then the idle TensorE sums
    over the class partitions with one-hot-column matmuls accumulating into
    PSUM[s, :] (super-tile s -> PSUM partition s). DVE ~2.2us + PE ~1.8us
    per super-tile (4 tiles), beating the 1x accum_out cap on DVE sums.
  - Final: ACT Ln over the two sum tensors with fused accumulation -> [P,2]
    partial lse sums per core; host combines.

Host computes the target-logit gather sum(x[i, t_i]) (0.1% of the data) and
the worst-class mask mean from the full-precision input:
  loss = (sum_lse - sum_gather)/B;  result = loss * (1 + mask_mean).

Error budget (validated in numpy + CoreSim): fp8 quantization is unbiased
(per-row lse std 3e-3); the bit-trick (c calibrated for the DVE's truncating
float->int16 store) biases lse < 1.5e-3; final relative error ~2.5e-4 vs the
2e-2 tolerance.
"""
import numpy as np
from contextlib import ExitStack

import concourse.bacc as bacc
import concourse.tile as tile
from concourse import mybir
from concourse.bass_utils import run_bass_kernel_spmd

F32 = mybir.dt.float32
BF16 = mybir.dt.bfloat16
I16 = mybir.dt.int16
F8 = mybir.dt.float8e4
AF = mybir.ActivationFunctionType
ALU = mybir.AluOpType

N_CORES = 8
B, C = 262144, 1000
ROWS = B // N_CORES           # 32768 rows per core
P = 128                       # SBUF partitions
CP = 1024                     # padded classes (8 chunks of 128)
R_SUP = 512                   # rows per super-tile (one PSUM bank column dim)

# Work split: N_CH_A chunks of KA [128 rows, C] tiles on ACT (exact exp),
# N_SUP transposed super-tiles of 512 rows on DVE+PE (bit-trick exp).
KA = 5
N_CH_A = 16
N_A = KA * N_CH_A             # 80 ACT tiles (10240 rows)
N_SUP = (ROWS - N_A * P) // R_SUP   # 44 super-tiles (22528 rows)
SUP_PER_DMA = 2               # super-tiles per xt DMA (1 MB transfers)
WORST = (3, 5, 8, 9)

# Schraudolph constants for bf16-bitspace exp: bits = x*A + (16256 - c).
# c calibrated for the truncating float->int16 store (CoreSim-verified).
A_TRICK = 128.0 / float(np.log(2.0))
B_TRICK = 16256.0 - 6.871
# Class padding value: exactly representable in fp8e4m3; trick bits land at
# ~1476 -> bf16 2.3e-35 ~= 0, and exp(-80) == 0 in f32, so pads add nothing.
X_PAD = -80.0

_CACHE = {}


def _build(reps: int = 1, ka: int = KA, n_ch_a: int = N_CH_A,
           n_sup: int = N_SUP, x_internal: bool = False):
    n_a = ka * n_ch_a
    nc = bacc.Bacc(None, target_bir_lowering=False, debug=False,
                   num_devices=N_CORES)

    def declare_x(name, shape, dtype):
        if not x_internal:
            return nc.declare_dram_parameter(name, shape, dtype, isOutput=False)
        # Timing-only builds: x as Internal DRAM (uninitialized) so repeated
        # timed calls carry no input upload. Output values are garbage.
        from concourse.bass import DRamTensorHandle
        nc._tensor(name, shape, dtype, kind="Internal", type="DRAM")
        return DRamTensorHandle(name, shape, dtype)

    xa_h = declare_x("xa", [n_ch_a, P, ka * C], F8) if n_a > 0 else None
    xt_h = declare_x("xt", [n_sup, P, 8 * R_SUP], F8) if n_sup > 0 else None
    out_h = nc.declare_dram_parameter("out", [P, 2], F32, isOutput=True)

    with tile.TileContext(nc) as tc, ExitStack() as ctx:
        xap = ctx.enter_context(tc.tile_pool(name="xap", bufs=3))
        xtp = ctx.enter_context(tc.tile_pool(name="xtp", bufs=4))
        scr = ctx.enter_context(tc.tile_pool(name="scr", bufs=6))
        pers = ctx.enter_context(tc.tile_pool(name="pers", bufs=1))
        pp = ctx.enter_context(tc.tile_pool(name="pp", bufs=1, space="PSUM"))

        s_a = pers.tile([P, max(n_a, 1)], F32, tag="s_a")
        fin = pers.tile([P, 2], F32, tag="fin")
        ps = pp.tile([64, R_SUP], F32, tag="ps")
        eye = pers.tile([P, 64 * 64], BF16, tag="eye")

        nc.vector.memset(fin[:], 0.0)
        # One-hot weight slabs: eye[:, i*64:(i+1)*64][p, m] = (m == i).
        # Built once on-device; slab i routes super-tile i's sum to PSUM row i.
        nc.vector.memset(eye[:], 0.0)
        for i in range(n_sup):
            nc.vector.memset(eye[:, i * 64 + i:i * 64 + i + 1], 1.0)

        def do_act_chunk(ch):
            xa_t = xap.tile([P, ka * C], F8, tag="xa_t")
            nc.sync.dma_start(out=xa_t[:], in_=xa_h[ch])
            for j in range(ka):
                k = ch * ka + j
                e_scr = scr.tile([P, C], BF16, tag="e_scr")
                nc.scalar.activation(
                    out=e_scr[:], in_=xa_t[:, j * C:(j + 1) * C],
                    func=AF.Exp, accum_out=s_a[:, k:k + 1],
                )

        def body():
            # Interleave the ACT chunk stream with the DVE/PE super-tile
            # stream so no engine starves at the ends; xt DMAs move
            # SUP_PER_DMA super-tiles (1 MB) for better HBM efficiency.
            n_dma = (n_sup + SUP_PER_DMA - 1) // SUP_PER_DMA if n_sup else 0
            for d in range(n_dma):
                for ch in range(d * n_ch_a // n_dma,
                                (d + 1) * n_ch_a // n_dma):
                    do_act_chunk(ch)
                s0 = d * SUP_PER_DMA
                cnt = min(SUP_PER_DMA, n_sup - s0)
                xt_big = xtp.tile([P, cnt * 8 * R_SUP], F8, tag="xt_t")
                nc.sync.dma_start(out=xt_big[:], in_=xt_h[s0:s0 + cnt])
                for si in range(cnt):
                    s = s0 + si
                    xt_t = xt_big[:, si * 8 * R_SUP:(si + 1) * 8 * R_SUP]
                    i_t = scr.tile([P, 8 * R_SUP], I16, tag="i_t")
                    nc.vector.tensor_scalar(
                        out=i_t[:], in0=xt_t, scalar1=A_TRICK, scalar2=B_TRICK,
                        op0=ALU.mult, op1=ALU.add,
                    )
                    for c in range(8):
                        nc.tensor.matmul(
                            out=ps[:, :],
                            lhsT=eye[:, s * 64:(s + 1) * 64],
                            rhs=i_t[:, c * R_SUP:(c + 1) * R_SUP].bitcast(BF16),
                            start=(s == 0 and c == 0),
                            stop=(s == n_sup - 1 and c == 7),
                        )
            if n_sup == 0:
                for ch in range(n_ch_a):
                    do_act_chunk(ch)

            if n_a > 0:
                ln_a = scr.tile([P, n_a], BF16, tag="ln_a")
                nc.scalar.activation(out=ln_a[:], in_=s_a[:, :n_a], func=AF.Ln,
                                     accum_out=fin[:, 0:1])
            if n_sup > 0:
                ln_p = scr.tile([64, R_SUP], BF16, tag="ln_p")
                nc.scalar.activation(out=ln_p[:n_sup], in_=ps[:n_sup, :],
                                     func=AF.Ln, accum_out=fin[:n_sup, 1:2])
            nc.sync.dma_start(out=out_h[:], in_=fin[:])

        if reps == 1:
            body()
        else:
            with tc.For_i(0, reps):
                body()

    nc.compile()
    return nc


def _shard_core(xs: np.ndarray, ka: int = KA, n_ch_a: int = N_CH_A,
                n_sup: int = N_SUP):
    """xs: [rows, C] f32 for one core -> {'xa': fp8 row-major, 'xt': fp8
    transposed super-tiles}."""
    import ml_dtypes
    f8 = ml_dtypes.float8_e4m3
    n_a = ka * n_ch_a
    rows_a = n_a * P
    m = {}
    if n_a > 0:
        m["xa"] = np.ascontiguousarray(
            xs[:rows_a].reshape(n_ch_a, P, ka * C)).astype(f8)
    if n_sup > 0:
        blocks = xs[rows_a:rows_a + n_sup * R_SUP].reshape(n_sup, R_SUP, C)
        padded = np.full((n_sup, R_SUP, CP), X_PAD, np.float32)
        padded[:, :, :C] = blocks
        # [s, r, cls] -> [s, p, c*R_SUP + r] with cls = c*128 + p
        xt = padded.transpose(0, 2, 1).reshape(n_sup, 8, P, R_SUP)
        xt = np.ascontiguousarray(xt.transpose(0, 2, 1, 3)).reshape(
            n_sup, P, 8 * R_SUP)
        m["xt"] = xt.astype(f8)
    return m


def _shard_inputs(output: np.ndarray):
    return [_shard_core(output[c * ROWS:(c + 1) * ROWS])
            for c in range(N_CORES)]


def _host_terms(output: np.ndarray, target: np.ndarray):
    g_sum = output[np.arange(B), target].astype(np.float64).sum()
    mask_mean = float(np.isin(target, np.asarray(WORST)).mean())
    return g_sum, mask_mean


def _combine(results, g_sum: float, mask_mean: float) -> np.float32:
    lse_sum = 0.0
    for r in results:
        fin = r["out"].astype(np.float64)
        lse_sum += fin[:, 0].sum() + fin[:N_SUP, 1].sum()
    loss = (lse_sum - g_sum) / B
    return np.float32(loss * (1.0 + mask_mean))


def _run(in_maps, **kwargs):
    if "nc" not in _CACHE:
        _CACHE["nc"] = _build()
    return run_bass_kernel_spmd(_CACHE["nc"], in_maps, list(range(N_CORES)),
                                **kwargs)


def kernel(output: np.ndarray, target: np.ndarray) -> np.float32:
    assert output.shape == (B, C) and target.shape == (B,)
    res = _run(_shard_inputs(output))
    g_sum, mask_mean = _host_terms(output, target)
    return _combine(res.results, g_sum, mask_mean)
